# revision 31
# baseline (speedup 1.0000x reference)
"""Trainium2 Bass kernel for LoRA self-attention (nn_LoRAAttnProcessor).

Problem shapes (hardcoded): x [2, 2048, 1280], 20 heads x 64 dim, LoRA rank 4.

Strategy
--------
* Host side: fold every LoRA pair into its base weight (W_eff = W + B @ A) and
  fold the 1/sqrt(D) score scale into Wq_eff.  Kernel computes plain MHA.
* Sharding: 8 cores x (batch b = core//4, 5 heads = core%4).  Wq/Wk/Wv
  column-sharded by head, Wo row-sharded; host sums 4 partial outputs per batch.
* Per core: attention runs as "pair passes" -- two 64-contraction score
  matmuls in distinct PE row groups (partitions 0:64 / 64:128) execute
  concurrently (tile_position row tiling).  Heads 0+1 and 2+3 pair up;
  head 4 pairs with itself across query halves using duplicated q4/k4
  feature rows (the A1 weight chunks that used to be zero padding).
* PSUM budget (8 banks): scores pool 2x[128,1024]f32 (4 banks, pair scores
  side by side -> one exp per tile), ctx pool 2x[128,512]f32 (2), proj pool
  2x[128,512]f32 (2).  The sk loop is software-pipelined one stage deep so
  the ACT-engine exp (~1147ns) paces it while PE fills slack with interleaved
  projection / output-projection matmuls (feeder).
* Softmax denominator rides as a 65th "ones" column of v; normalization uses
  reciprocal + a PE broadcast (ones[1,64] matmul) instead of a DRAM bounce.
"""

import sys

if "/opt/trn_rl_repo" not in sys.path:
    sys.path.insert(0, "/opt/trn_rl_repo")

from contextlib import ExitStack

import ml_dtypes
import numpy as np

import concourse.bass as bass
import concourse.tile as tile
from concourse import bacc, mybir
from concourse.bass_utils import run_bass_kernel_spmd

BF16 = mybir.dt.bfloat16
F32 = mybir.dt.float32
NPBF16 = ml_dtypes.bfloat16

D = 64
H_LOC = 5  # heads per core
N_CORES = 8


def build_program(S=2048, C=1280, repeat=1):
    """SPMD single-core program. S % 1024 == 0, C % 128 == 0."""
    assert S % 1024 == 0 and C % 128 == 0
    CK = C // 128          # contraction chunks over channels
    SM = S // 128          # 128-row chunks of sequence
    SK = S // 128          # key chunks
    NS4 = S // 512         # 512-col blocks of sequence

    nc = bacc.Bacc("TRN2", target_bir_lowering=False, debug=False)

    xT_d = nc.dram_tensor("xT", [C, S], BF16, kind="ExternalInput").ap()
    wqk_d = nc.dram_tensor("wqk", [C, 768], BF16, kind="ExternalInput").ap()
    wvT_d = nc.dram_tensor("wvT", [C, H_LOC * D], BF16, kind="ExternalInput").ap()
    woT_d = nc.dram_tensor("woT", [384, C], BF16, kind="ExternalInput").ap()
    out_d = nc.dram_tensor("out_part", [S, C], BF16, kind="ExternalOutput").ap()

    EXP = mybir.ActivationFunctionType.Exp
    MULT = mybir.AluOpType.mult

    with tile.TileContext(nc) as tc, ExitStack() as ctx:
        persist = ctx.enter_context(tc.tile_pool(name="persist", bufs=1))
        psc = ctx.enter_context(tc.tile_pool(name="psc", bufs=2, space="PSUM"))
        pctx = ctx.enter_context(tc.tile_pool(name="pctx", bufs=2, space="PSUM"))
        pproj = ctx.enter_context(tc.tile_pool(name="pproj", bufs=2, space="PSUM"))
        ppool = ctx.enter_context(tc.tile_pool(name="probs", bufs=5))
        smallp = ctx.enter_context(tc.tile_pool(name="small", bufs=4))
        outp = ctx.enter_context(tc.tile_pool(name="osb", bufs=4))
        dramp = ctx.enter_context(tc.tile_pool(name="scratch", bufs=2, space="DRAM"))

        xT_sb = persist.tile([128, CK, S], BF16, tag="xT")
        wqk_sb = persist.tile([128, CK, 768], BF16, tag="wqk")
        wvT_sb = persist.tile([128, CK, H_LOC * D], BF16, tag="wvT")
        woT_sb = persist.tile([128, 3, C], BF16, tag="woT")
        qkT_full = persist.tile([128, 2, 6, S], BF16, tag="qkT")
        v_full = persist.tile([128, 2, SM, H_LOC, D + 1], BF16, tag="vsb")
        ctxT_full = persist.tile([128, 2, 3, S], BF16, tag="ctxT")
        ones_sb = persist.tile([1, D], BF16, tag="ones")

        def emit_body(rep, carry_in):
            par = rep % 2
            qkT_sb = qkT_full[:, par]
            v_sb = v_full[:, par]
            ctxT_sb = ctxT_full[:, par]
            # chunked input loads: A1's c-loop can start after the first
            # (wqk, xT) chunk pair lands instead of the full 5MB xT DMA.
            wqk_r = wqk_d.rearrange("(o p) n -> p o n", p=128)
            xT_r = xT_d.rearrange("(o p) n -> p o n", p=128)
            wvT_r = wvT_d.rearrange("(o p) n -> p o n", p=128)
            for c in range(CK):
                nc.sync.dma_start(wqk_sb[:, c], wqk_r[:, c])
                nc.sync.dma_start(xT_sb[:, c], xT_r[:, c])
            for c in range(CK):
                nc.sync.dma_start(wvT_sb[:, c], wvT_r[:, c])
            nc.sync.dma_start(woT_sb[:], woT_d.rearrange("(o p) n -> p o n", p=128))

            nc.vector.memset(v_sb[:, :, :, D : D + 1], 1.0)
            nc.vector.memset(ctxT_sb[64:128, 2, :], 0.0)
            nc.vector.memset(ones_sb[:], 1.0)

            # ---------------- feeder: PE filler work -----------------------
            # Thunks each emit ~one matmul (est_ns, fn); the attention loop
            # drains them against a per-iteration PE-slack budget so the ACT
            # exp cadence is never starved by long PE bursts.
            feed_queue = []
            feed_credit = [0.0]

            def feed(budget_ns):
                feed_credit[0] += budget_ns
                while feed_queue and feed_credit[0] >= feed_queue[0][0]:
                    est, fn = feed_queue.pop(0)
                    feed_credit[0] -= est
                    fn()

            def feed_all():
                while feed_queue:
                    feed_queue.pop(0)[1]()
                feed_credit[0] = 0.0

            def a1_group(f, s4):
                state = {}

                def half(h):
                    def fn():
                        if h == 0:
                            state["ps"] = pproj.tile(
                                [128, 512], F32, tag="pj", name=f"a1_{f}_{s4}"
                            )
                        for c in range(h * CK // 2, (h + 1) * CK // 2):
                            nc.tensor.matmul(
                                state["ps"][:],
                                lhsT=wqk_sb[:, c, f * 128 : (f + 1) * 128],
                                rhs=xT_sb[:, c, s4 * 512 : (s4 + 1) * 512],
                                start=(c == 0),
                                stop=(c == CK - 1),
                            )
                        if h == 1:
                            nc.vector.tensor_copy(
                                out=qkT_sb[:, f, s4 * 512 : (s4 + 1) * 512],
                                in_=state["ps"][:],
                            )
                    return (CK // 2 * 213.0, fn)

                return [half(0), half(1)]

            def a2_group(m):
                def thunk():
                    ps = pproj.tile([128, 512], F32, tag="pj", name=f"a2_{m}")
                    for c in range(CK):
                        nc.tensor.matmul(
                            ps[:, 0 : H_LOC * D],
                            lhsT=xT_sb[:, c, m * 128 : (m + 1) * 128],
                            rhs=wvT_sb[:, c, :],
                            start=(c == 0),
                            stop=(c == CK - 1),
                        )
                    nc.vector.tensor_copy(
                        out=v_sb[:, m, :, 0:D],
                        in_=ps[:, 0 : H_LOC * D].rearrange(
                            "p (h d) -> p h d", h=H_LOC
                        ),
                    )
                return thunk

            def oproj_group(m, ctxT_src):
                # j order (0, 2, 1): the j=1 chunk (heads 2,3) depends on the
                # last-finishing normalize, so accumulate it last.
                state = {}
                cols = [(c0, min(512, C - c0)) for c0 in range(0, C, 512)]
                jseq = (0, 2, 1)

                def mm(ci, jj):
                    col0, w = cols[ci]
                    j = jseq[jj]

                    def fn():
                        if ci == 0 and jj == 0:
                            state["os"] = outp.tile(
                                [128, C], BF16, tag="osb", name=f"os_{m}"
                            )
                        if jj == 0:
                            state["ps"] = pproj.tile(
                                [128, 512], F32, tag="pj", name=f"op_{m}_{col0}"
                            )
                        nc.tensor.matmul(
                            state["ps"][:, 0:w],
                            lhsT=ctxT_src[:, j, m * 128 : (m + 1) * 128],
                            rhs=woT_sb[:, j, col0 : col0 + w],
                            start=(jj == 0),
                            stop=(jj == 2),
                        )
                        if jj == 2:
                            nc.vector.tensor_copy(
                                out=state["os"][:, col0 : col0 + w],
                                in_=state["ps"][:, 0:w],
                            )
                        if ci == len(cols) - 1 and jj == 2:
                            nc.sync.dma_start(
                                out_d[m * 128 : (m + 1) * 128, :], state["os"][:]
                            )
                    return (w * 0.417 + 20, fn)

                return [mm(ci, jj) for ci in range(len(cols)) for jj in range(3)]

            # ---------------- attention pair pass --------------------------
            # lanes: (row_off, kc, qc, q_col_base, v_head, ctx_jc, ctx_po)
            def attn_pass(lanes, q0, name):
                """One 512-query-wide pass over all SK key chunks for 2 lanes."""
                ctxs = [
                    pctx.tile([128, 512], F32, tag="ctx", name=f"c_{name}_{li}")
                    for li in range(2)
                ]
                pt_prev = None
                for sk in range(SK + 1):
                    if sk < SK:
                        sc = psc.tile([128, 1024], F32, tag="sc", name=f"s_{name}_{sk}")
                        for li, (ro, kc, qc, qb, vh, jc, po) in enumerate(lanes):
                            nc.tensor.matmul(
                                sc[:, li * 512 : (li + 1) * 512],
                                lhsT=qkT_sb[ro : ro + D, kc, sk * 128 : (sk + 1) * 128],
                                rhs=qkT_sb[ro : ro + D, qc, qb + q0 : qb + q0 + 512],
                                start=True,
                                stop=True,
                            )
                        pt = ppool.tile([128, 1024], BF16, tag="probs", name=f"p_{name}_{sk}")
                        nc.scalar.activation(pt[:], sc[:], EXP)
                        feed(500.0)
                    if sk > 0:
                        skm = sk - 1
                        for li, (ro, kc, qc, qb, vh, jc, po) in enumerate(lanes):
                            nc.tensor.matmul(
                                ctxs[li][0 : D + 1, :],
                                lhsT=v_sb[:, skm, vh, :],
                                rhs=pt_prev[:, li * 512 : (li + 1) * 512],
                                start=(skm == 0),
                                stop=(skm == SK - 1),
                            )
                    pt_prev = pt
                # normalize: ctxT = stage[0:64] * recip(stage[64]).  Stage 1
                # (emitted NOW): evacuate the ctx psum tile to SBUF so the
                # pctx slot frees immediately, fire recip + the DRAM-bounce
                # broadcast DMAs.  Stage 2 (deferred into the feeder queue):
                # the multiply, by when the bounce DMAs have landed.
                stages = []
                for li, (ro, kc, qc, qb, vh, jc, po) in enumerate(lanes):
                    stage = smallp.tile(
                        [D + 1, 512], F32, tag="stg", name=f"g_{name}_{li}"
                    )
                    nc.vector.tensor_copy(out=stage[:], in_=ctxs[li][0 : D + 1, :])
                    rec = smallp.tile([1, 512], F32, tag="rec", name=f"r_{name}_{li}")
                    nc.vector.reciprocal(rec[:], stage[D : D + 1, :])
                    scr = dramp.tile([1, 512], F32, name=f"sc_{name}_{li}")
                    nc.sync.dma_start(scr[:], rec[:])
                    bcs = smallp.tile([D, 512], F32, tag="bcs", name=f"bs_{name}_{li}")
                    nc.sync.dma_start(bcs[:], scr[:].to_broadcast((D, 512)))
                    stages.append((stage, bcs))

                def mult_thunk(li, jc, po, qb):
                    stage, bcs = stages[li]

                    def fn():
                        nc.vector.tensor_tensor(
                            out=ctxT_sb[po : po + D, jc, qb + q0 : qb + q0 + 512],
                            in0=stage[0:D, :],
                            in1=bcs[:],
                            op=MULT,
                        )
                    return (50.0, fn)

                for li, (ro, kc, qc, qb, vh, jc, po) in enumerate(lanes):
                    feed_queue.insert(
                        min(4 + li, len(feed_queue)), mult_thunk(li, jc, po, qb)
                    )

            # ---------------- schedule -------------------------------------
            # A1 chunk layout: 0=q0q1 1=q2q3 2=k0k1 3=k2k3 4=q4|q4 5=k4|k4
            # head h<4: q rows at (h//2, (h%2)*64), k at (2+h//2, (h%2)*64).
            # ctxT row of head h: jc=h*64//128, po=(h*64)%128.
            # Front: only what pass 0 reads from its first iteration — all of
            # k4 (chunk 5), q4 cols for pass 0 (s4 0, 2), and all of v.
            # Remaining A1 groups feed during passes ahead of their deadline.
            for f in (4, 5):
                for s4 in range(NS4):
                    for _, t in a1_group(f, s4):
                        t()
            for m in range(SM):
                a2_group(m)()

            # Pass sequence interleaves the pairs so out-proj query ranges
            # unlock early: Q0 is complete after pass 5 (h4A covers Q0&Q2,
            # h4B covers Q1&Q3).  A1 chunks feed during the preceding passes.
            h4_lanes = [
                (0, 5, 4, 0, 4, 2, 0),
                (64, 5, 4, 1024, 4, 2, 0),
            ]
            p01 = [
                (0, 2, 0, 0, 0, 0, 0),
                (64, 2, 0, 0, 1, 0, 64),
            ]
            p23 = [
                (0, 3, 1, 0, 2, 1, 0),
                (64, 3, 1, 0, 3, 1, 64),
            ]
            seq = [
                (h4_lanes, 0, "h4_0"),      # covers Q0, Q2
                (h4_lanes, 512, "h4_512"),  # covers Q1, Q3
                (p01, 0, "p01_0"),
                (p01, 512, "p01_512"),
                (p23, 0, "p23_0"),          # Q0 complete after this
                (p01, 1024, "p01_1024"),
                (p23, 512, "p23_512"),      # Q1 complete
                (p01, 1536, "p01_1536"),
                (p23, 1024, "p23_1024"),    # Q2 complete
                (p23, 1536, "p23_1536"),    # Q3 complete
            ]
            # feeder unlock schedule: before pass index i runs, queue work in
            # deadline order (earliest-needed first).  q-chunk groups (0, 1,
            # 4) are consumed per 512-query block, k-chunks (2, 3, 5) whole.
            unlock = {
                0: [("a1", f, s4) for f in (0, 2) for s4 in range(NS4)],
                2: [("a1", f, s4) for f in (1, 3) for s4 in range(NS4)],
                5: [("op", m) for m in range(0, 4)],
                7: [("op", m) for m in range(4, 8)],
                9: [("op", m) for m in range(8, 12)],
            }
            for pi, (lanes, q0, name) in enumerate(seq):
                if pi == 2 and carry_in:
                    feed_queue.extend(carry_in)
                    carry_in.clear()
                for item in unlock.get(pi, []):
                    if item[0] == "a1":
                        feed_queue.extend(a1_group(item[1], item[2]))
                    else:
                        feed_queue.extend(oproj_group(item[1], ctxT_sb))
                attn_pass(lanes, q0, name)
            if carry_in:
                feed_queue.extend(carry_in)
                carry_in.clear()
            feed_all()
            # tail out-proj: deferred into the next body's feeder (the parity
            # ctxT buffer keeps it valid); the final body flushes it below.
            tail = []
            for m in range(12, SM):
                tail.extend(oproj_group(m, ctxT_sb))
            return tail

        carry = []
        for rep in range(repeat):
            carry = emit_body(rep, carry)
        for _, t in carry:
            t()

    nc.compile()
    return nc


def make_core_inputs(x, Wq_eff, Wk_eff, Wv_eff, Wo_eff):
    """Per-core input dicts. x [B,S,C] f32; W_eff [C,C] f32 (scale folded)."""
    B, S, C = x.shape
    in_maps = []
    xT16 = [np.ascontiguousarray(x[b].T).astype(NPBF16) for b in range(B)]
    for core in range(N_CORES):
        b, g = core // 4, core % 4
        r0 = g * H_LOC * D  # first feature row of this core's heads
        qf = Wq_eff[r0 : r0 + H_LOC * D]  # (320, C)
        kf = Wk_eff[r0 : r0 + H_LOC * D]
        vf = Wv_eff[r0 : r0 + H_LOC * D]
        # chunks: (q0,q1)(q2,q3)(k0,k1)(k2,k3)(q4,q4)(k4,k4)
        wqk = np.concatenate(
            [
                qf[: 4 * D],
                kf[: 4 * D],
                qf[4 * D :],
                qf[4 * D :],
                kf[4 * D :],
                kf[4 * D :],
            ],
            axis=0,
        ).T  # (C, 768)
        wvT = vf.T  # (C, 320)
        woT = np.concatenate(
            [Wo_eff[:, r0 : r0 + H_LOC * D].T, np.zeros((D, C), np.float32)], axis=0
        )  # (384, C)
        in_maps.append(
            {
                "xT": xT16[b],
                "wqk": np.ascontiguousarray(wqk).astype(NPBF16),
                "wvT": np.ascontiguousarray(wvT).astype(NPBF16),
                "woT": np.ascontiguousarray(woT).astype(NPBF16),
            }
        )
    return in_maps


def fold_weights(Wq, Wk, Wv, Wo, Aq, Bq, Ak, Bk, Av, Bv, Ao, Bo):
    scale = 1.0 / np.sqrt(np.float32(D))
    Wq_eff = (Wq + Bq @ Aq) * scale
    Wk_eff = Wk + Bk @ Ak
    Wv_eff = Wv + Bv @ Av
    Wo_eff = Wo + Bo @ Ao
    return Wq_eff, Wk_eff, Wv_eff, Wo_eff


_NC_CACHE = {}


def _get_program(S, C):
    key = (S, C)
    if key not in _NC_CACHE:
        _NC_CACHE[key] = build_program(S, C)
    return _NC_CACHE[key]


def kernel(**inputs):
    inputs = {k: np.asarray(v, np.float32) for k, v in inputs.items()}
    x = inputs["x"]
    B, S, C = x.shape
    Wq_eff, Wk_eff, Wv_eff, Wo_eff = fold_weights(
        inputs["Wq"], inputs["Wk"], inputs["Wv"], inputs["Wo"],
        inputs["Aq"], inputs["Bq"], inputs["Ak"], inputs["Bk"],
        inputs["Av"], inputs["Bv"], inputs["Ao"], inputs["Bo"],
    )
    in_maps = make_core_inputs(x, Wq_eff, Wk_eff, Wv_eff, Wo_eff)
    nc = _get_program(S, C)
    res = run_bass_kernel_spmd(nc, in_maps, list(range(N_CORES)))
    parts = [res.results[c]["out_part"].astype(np.float32) for c in range(N_CORES)]
    bo = inputs["bo"]
    out = np.stack(
        [
            parts[0] + parts[1] + parts[2] + parts[3] + bo,
            parts[4] + parts[5] + parts[6] + parts[7] + bo,
        ]
    ).astype(np.float32)
    return out


# revision 37
# speedup vs baseline: 1.1711x; 1.1711x over previous
"""Trainium2 Bass kernel for LoRA self-attention (nn_LoRAAttnProcessor).

Problem shapes (hardcoded): x [2, 2048, 1280], 20 heads x 64 dim, LoRA rank 4.

Strategy
--------
* Host side: fold every LoRA pair into its base weight (W_eff = W + B @ A) and
  fold the 1/sqrt(D) score scale into Wq_eff.  Kernel computes plain MHA.
* Sharding: 8 cores x (batch b = core//4, 5 heads = core%4).  Wq/Wk/Wv
  column-sharded by head, Wo row-sharded; host sums 4 partial outputs per batch.
* Per core: attention runs as "pair passes" -- two 64-contraction score
  matmuls in distinct PE row groups (partitions 0:64 / 64:128) execute
  concurrently (tile_position row tiling).  Heads 0+1 and 2+3 pair up;
  head 4 pairs with itself across query halves using duplicated q4/k4
  feature rows (the A1 weight chunks that used to be zero padding).
* PSUM budget (8 banks): scores pool 2x[128,1024]f32 (4 banks, pair scores
  side by side -> one exp per tile), ctx pool 2x[128,512]f32 (2), proj pool
  2x[128,512]f32 (2).  The sk loop is software-pipelined one stage deep so
  the ACT-engine exp (~1us) paces it while PE fills slack with interleaved
  projection / output-projection matmuls fed at ~1-matmul granularity
  against a per-iteration ns budget (feeder).
* Softmax denominator rides as a 65th "ones" column of v.  At pass end the
  ctx psum tile is immediately evacuated to SBUF (frees the bank), the
  reciprocal row takes a DRAM-bounce broadcast, and the normalize multiply
  is deferred a few feeder slots so the DMA latency stays off every engine's
  critical path.  Tail out-projection defers into the next repeat body's
  feeder (qkT/v/ctxT are parity double-buffered across bodies).
"""

import sys

if "/opt/trn_rl_repo" not in sys.path:
    sys.path.insert(0, "/opt/trn_rl_repo")

from contextlib import ExitStack

import ml_dtypes
import numpy as np

import concourse.bass as bass
import concourse.tile as tile
from concourse import bacc, mybir
from concourse.bass_utils import run_bass_kernel_spmd

BF16 = mybir.dt.bfloat16
F32 = mybir.dt.float32
NPBF16 = ml_dtypes.bfloat16

D = 64
H_LOC = 5  # heads per core
N_CORES = 8


def build_program(S=2048, C=1280, repeat=1):
    """SPMD single-core program. S % 1024 == 0, C % 128 == 0."""
    assert S % 1024 == 0 and C % 128 == 0
    CK = C // 128          # contraction chunks over channels
    SM = S // 128          # 128-row chunks of sequence
    SK = S // 128          # key chunks
    NS4 = S // 512         # 512-col blocks of sequence

    nc = bacc.Bacc("TRN2", target_bir_lowering=False, debug=False)

    xT_d = nc.dram_tensor("xT", [C, S], BF16, kind="ExternalInput").ap()
    wqk_d = nc.dram_tensor("wqk", [C, 768], BF16, kind="ExternalInput").ap()
    wvT_d = nc.dram_tensor("wvT", [C, H_LOC * D], BF16, kind="ExternalInput").ap()
    woT_d = nc.dram_tensor("woT", [384, C], BF16, kind="ExternalInput").ap()
    out_d = nc.dram_tensor("out_part", [S, C], BF16, kind="ExternalOutput").ap()

    EXP = mybir.ActivationFunctionType.Exp
    MULT = mybir.AluOpType.mult

    with tile.TileContext(nc) as tc, ExitStack() as ctx:
        persist = ctx.enter_context(tc.tile_pool(name="persist", bufs=1))
        psc = ctx.enter_context(tc.tile_pool(name="psc", bufs=2, space="PSUM"))
        pctx = ctx.enter_context(tc.tile_pool(name="pctx", bufs=2, space="PSUM"))
        pproj = ctx.enter_context(tc.tile_pool(name="pproj", bufs=2, space="PSUM"))
        ppool = ctx.enter_context(tc.tile_pool(name="probs", bufs=5))
        smallp = ctx.enter_context(tc.tile_pool(name="small", bufs=4))
        outp = ctx.enter_context(tc.tile_pool(name="osb", bufs=4))
        dramp = ctx.enter_context(tc.tile_pool(name="scratch", bufs=6, space="DRAM"))

        xT_sb = persist.tile([128, CK, S], BF16, tag="xT")
        wqk_sb = persist.tile([128, CK, 768], BF16, tag="wqk")
        wvT_sb = persist.tile([128, CK, H_LOC * D], BF16, tag="wvT")
        woT_sb = persist.tile([128, 3, C], BF16, tag="woT")
        qkT_full = persist.tile([128, 2, 6, S], BF16, tag="qkT")
        v_full = persist.tile([128, 2, SM, H_LOC, D + 1], BF16, tag="vsb")
        ctxT_full = persist.tile([128, 2, 3, S], BF16, tag="ctxT")
        ones_sb = persist.tile([1, D], BF16, tag="ones")

        def emit_body(rep, carry_in):
            par = rep % 2
            qkT_sb = qkT_full[:, par]
            v_sb = v_full[:, par]
            ctxT_sb = ctxT_full[:, par]
            # chunked input loads: A1's c-loop can start after the first
            # (wqk, xT) chunk pair lands instead of the full 5MB xT DMA.
            wqk_r = wqk_d.rearrange("(o p) n -> p o n", p=128)
            xT_r = xT_d.rearrange("(o p) n -> p o n", p=128)
            wvT_r = wvT_d.rearrange("(o p) n -> p o n", p=128)
            for c in range(CK):
                nc.sync.dma_start(wqk_sb[:, c], wqk_r[:, c])
                nc.sync.dma_start(xT_sb[:, c], xT_r[:, c])
            for c in range(CK):
                nc.sync.dma_start(wvT_sb[:, c], wvT_r[:, c])
            nc.sync.dma_start(woT_sb[:], woT_d.rearrange("(o p) n -> p o n", p=128))

            nc.vector.memset(v_sb[:, :, :, D : D + 1], 1.0)
            nc.vector.memset(ctxT_sb[64:128, 2, :], 0.0)
            nc.vector.memset(ones_sb[:], 1.0)

            # ---------------- feeder: PE filler work -----------------------
            # Thunks each emit ~one matmul (est_ns, fn); the attention loop
            # drains them against a per-iteration PE-slack budget so the ACT
            # exp cadence is never starved by long PE bursts.
            feed_queue = []
            feed_credit = [0.0]

            def feed(budget_ns):
                feed_credit[0] += budget_ns
                while feed_queue and feed_credit[0] >= feed_queue[0][0]:
                    est, fn = feed_queue.pop(0)
                    feed_credit[0] -= est
                    fn()

            def feed_all():
                while feed_queue:
                    feed_queue.pop(0)[1]()
                feed_credit[0] = 0.0

            def a1_group(f, s4):
                state = {}

                def half(h):
                    def fn():
                        if h == 0:
                            state["ps"] = pproj.tile(
                                [128, 512], F32, tag="pj", name=f"a1_{f}_{s4}"
                            )
                        for c in range(h * CK // 2, (h + 1) * CK // 2):
                            nc.tensor.matmul(
                                state["ps"][:],
                                lhsT=wqk_sb[:, c, f * 128 : (f + 1) * 128],
                                rhs=xT_sb[:, c, s4 * 512 : (s4 + 1) * 512],
                                start=(c == 0),
                                stop=(c == CK - 1),
                            )
                        if h == 1:
                            nc.vector.tensor_copy(
                                out=qkT_sb[:, f, s4 * 512 : (s4 + 1) * 512],
                                in_=state["ps"][:],
                            )
                    return (CK // 2 * 213.0, fn)

                return [half(0), half(1)]

            def a2_group(m):
                def thunk():
                    ps = pproj.tile([128, 512], F32, tag="pj", name=f"a2_{m}")
                    for c in range(CK):
                        nc.tensor.matmul(
                            ps[:, 0 : H_LOC * D],
                            lhsT=xT_sb[:, c, m * 128 : (m + 1) * 128],
                            rhs=wvT_sb[:, c, :],
                            start=(c == 0),
                            stop=(c == CK - 1),
                        )
                    nc.vector.tensor_copy(
                        out=v_sb[:, m, :, 0:D],
                        in_=ps[:, 0 : H_LOC * D].rearrange(
                            "p (h d) -> p h d", h=H_LOC
                        ),
                    )
                return thunk

            def oproj_group(m, ctxT_src):
                # j order (0, 2, 1): the j=1 chunk (heads 2,3) depends on the
                # last-finishing normalize, so accumulate it last.
                state = {}
                cols = [(c0, min(512, C - c0)) for c0 in range(0, C, 512)]
                jseq = (0, 2, 1)

                def mm(ci, jj):
                    col0, w = cols[ci]
                    j = jseq[jj]

                    def fn():
                        if ci == 0 and jj == 0:
                            state["os"] = outp.tile(
                                [128, C], BF16, tag="osb", name=f"os_{m}"
                            )
                        if jj == 0:
                            state["ps"] = pproj.tile(
                                [128, 512], F32, tag="pj", name=f"op_{m}_{col0}"
                            )
                        nc.tensor.matmul(
                            state["ps"][:, 0:w],
                            lhsT=ctxT_src[:, j, m * 128 : (m + 1) * 128],
                            rhs=woT_sb[:, j, col0 : col0 + w],
                            start=(jj == 0),
                            stop=(jj == 2),
                        )
                        if jj == 2:
                            nc.vector.tensor_copy(
                                out=state["os"][:, col0 : col0 + w],
                                in_=state["ps"][:, 0:w],
                            )
                        if ci == len(cols) - 1 and jj == 2:
                            nc.sync.dma_start(
                                out_d[m * 128 : (m + 1) * 128, :], state["os"][:]
                            )
                    return (w * 0.417 + 20, fn)

                return [mm(ci, jj) for ci in range(len(cols)) for jj in range(3)]

            # ---------------- attention pair pass --------------------------
            # lanes: (row_off, kc, qc, q_col_base, v_head, ctx_jc, ctx_po)
            def attn_pass(lanes, q0, name):
                """One 512-query-wide pass over all SK key chunks for 2 lanes."""
                ctxs = [
                    pctx.tile([128, 512], F32, tag="ctx", name=f"c_{name}_{li}")
                    for li in range(2)
                ]
                pt_prev = None
                for sk in range(SK + 1):
                    if sk < SK:
                        sc = psc.tile([128, 1024], F32, tag="sc", name=f"s_{name}_{sk}")
                        for li, (ro, kc, qc, qb, vh, jc, po) in enumerate(lanes):
                            nc.tensor.matmul(
                                sc[:, li * 512 : (li + 1) * 512],
                                lhsT=qkT_sb[ro : ro + D, kc, sk * 128 : (sk + 1) * 128],
                                rhs=qkT_sb[ro : ro + D, qc, qb + q0 : qb + q0 + 512],
                                start=True,
                                stop=True,
                            )
                        pt = ppool.tile([128, 1024], BF16, tag="probs", name=f"p_{name}_{sk}")
                        nc.scalar.activation(pt[:], sc[:], EXP)
                        feed(500.0)
                    if sk > 0:
                        skm = sk - 1
                        for li, (ro, kc, qc, qb, vh, jc, po) in enumerate(lanes):
                            nc.tensor.matmul(
                                ctxs[li][0 : D + 1, :],
                                lhsT=v_sb[:, skm, vh, :],
                                rhs=pt_prev[:, li * 512 : (li + 1) * 512],
                                start=(skm == 0),
                                stop=(skm == SK - 1),
                            )
                    pt_prev = pt
                # normalize: ctxT = stage[0:64] * recip(stage[64]).  Stage 1
                # (emitted NOW): evacuate the ctx psum tile to SBUF so the
                # pctx slot frees immediately, fire recip + the DRAM-bounce
                # broadcast DMAs.  Stage 2 (deferred into the feeder queue):
                # the multiply, by when the bounce DMAs have landed.
                stages = []
                for li, (ro, kc, qc, qb, vh, jc, po) in enumerate(lanes):
                    stage = smallp.tile(
                        [D + 1, 512], F32, tag="stg", name=f"g_{name}_{li}"
                    )
                    nc.vector.tensor_copy(out=stage[:], in_=ctxs[li][0 : D + 1, :])
                    rec = smallp.tile([1, 512], F32, tag="rec", name=f"r_{name}_{li}")
                    nc.vector.reciprocal(rec[:], stage[D : D + 1, :])
                    scr = dramp.tile([1, 512], F32, name=f"sc_{name}_{li}")
                    nc.sync.dma_start(scr[:], rec[:])
                    bcs = smallp.tile([D, 512], F32, tag="bcs", name=f"bs_{name}_{li}")
                    nc.sync.dma_start(bcs[:], scr[:].to_broadcast((D, 512)))
                    stages.append((stage, bcs))

                def mult_thunk(li, jc, po, qb):
                    stage, bcs = stages[li]

                    def fn():
                        nc.vector.tensor_tensor(
                            out=ctxT_sb[po : po + D, jc, qb + q0 : qb + q0 + 512],
                            in0=stage[0:D, :],
                            in1=bcs[:],
                            op=MULT,
                        )
                    return (50.0, fn)

                for li, (ro, kc, qc, qb, vh, jc, po) in enumerate(lanes):
                    feed_queue.insert(
                        min(4 + li, len(feed_queue)), mult_thunk(li, jc, po, qb)
                    )

            # ---------------- schedule -------------------------------------
            # A1 chunk layout: 0=q0q1 1=q2q3 2=k0k1 3=k2k3 4=q4|q4 5=k4|k4
            # head h<4: q rows at (h//2, (h%2)*64), k at (2+h//2, (h%2)*64).
            # ctxT row of head h: jc=h*64//128, po=(h*64)%128.
            # Front: only what pass 0 reads from its first iteration — all of
            # k4 (chunk 5), q4 cols for pass 0 (s4 0, 2), and all of v.
            # Remaining A1 groups feed during passes ahead of their deadline.
            for f in (4, 5):
                for s4 in range(NS4):
                    for _, t in a1_group(f, s4):
                        t()
            for m in range(SM):
                a2_group(m)()

            # Pass sequence interleaves the pairs so out-proj query ranges
            # unlock early: Q0 is complete after pass 5 (h4A covers Q0&Q2,
            # h4B covers Q1&Q3).  A1 chunks feed during the preceding passes.
            h4_lanes = [
                (0, 5, 4, 0, 4, 2, 0),
                (64, 5, 4, 1024, 4, 2, 0),
            ]
            p01 = [
                (0, 2, 0, 0, 0, 0, 0),
                (64, 2, 0, 0, 1, 0, 64),
            ]
            p23 = [
                (0, 3, 1, 0, 2, 1, 0),
                (64, 3, 1, 0, 3, 1, 64),
            ]
            seq = [
                (h4_lanes, 0, "h4_0"),      # covers Q0, Q2
                (h4_lanes, 512, "h4_512"),  # covers Q1, Q3
                (p01, 0, "p01_0"),
                (p01, 512, "p01_512"),
                (p23, 0, "p23_0"),          # Q0 complete after this
                (p01, 1024, "p01_1024"),
                (p23, 512, "p23_512"),      # Q1 complete
                (p01, 1536, "p01_1536"),
                (p23, 1024, "p23_1024"),    # Q2 complete
                (p23, 1536, "p23_1536"),    # Q3 complete
            ]
            # feeder unlock schedule: before pass index i runs, queue work in
            # deadline order (earliest-needed first).  q-chunk groups (0, 1,
            # 4) are consumed per 512-query block, k-chunks (2, 3, 5) whole.
            unlock = {
                0: [("a1", f, s4) for f in (0, 2) for s4 in range(NS4)],
                2: [("a1", f, s4) for f in (1, 3) for s4 in range(NS4)],
                5: [("op", m) for m in range(0, 4)],
                7: [("op", m) for m in range(4, 8)],
                9: [("op", m) for m in range(8, 12)],
            }
            for pi, (lanes, q0, name) in enumerate(seq):
                if pi == 2 and carry_in:
                    feed_queue.extend(carry_in)
                    carry_in.clear()
                for item in unlock.get(pi, []):
                    if item[0] == "a1":
                        feed_queue.extend(a1_group(item[1], item[2]))
                    else:
                        feed_queue.extend(oproj_group(item[1], ctxT_sb))
                attn_pass(lanes, q0, name)
            if carry_in:
                feed_queue.extend(carry_in)
                carry_in.clear()
            feed_all()
            # tail out-proj: deferred into the next body's feeder (the parity
            # ctxT buffer keeps it valid); the final body flushes it below.
            tail = []
            for m in range(12, SM):
                tail.extend(oproj_group(m, ctxT_sb))
            return tail

        carry = []
        for rep in range(repeat):
            carry = emit_body(rep, carry)
        for _, t in carry:
            t()

    nc.compile()
    return nc


def make_core_inputs(x, Wq_eff, Wk_eff, Wv_eff, Wo_eff):
    """Per-core input dicts. x [B,S,C] f32; W_eff [C,C] f32 (scale folded)."""
    B, S, C = x.shape
    in_maps = []
    xT16 = [np.ascontiguousarray(x[b].T).astype(NPBF16) for b in range(B)]
    for core in range(N_CORES):
        b, g = core // 4, core % 4
        r0 = g * H_LOC * D  # first feature row of this core's heads
        qf = Wq_eff[r0 : r0 + H_LOC * D]  # (320, C)
        kf = Wk_eff[r0 : r0 + H_LOC * D]
        vf = Wv_eff[r0 : r0 + H_LOC * D]
        # chunks: (q0,q1)(q2,q3)(k0,k1)(k2,k3)(q4,q4)(k4,k4)
        wqk = np.concatenate(
            [
                qf[: 4 * D],
                kf[: 4 * D],
                qf[4 * D :],
                qf[4 * D :],
                kf[4 * D :],
                kf[4 * D :],
            ],
            axis=0,
        ).T  # (C, 768)
        wvT = vf.T  # (C, 320)
        woT = np.concatenate(
            [Wo_eff[:, r0 : r0 + H_LOC * D].T, np.zeros((D, C), np.float32)], axis=0
        )  # (384, C)
        in_maps.append(
            {
                "xT": xT16[b],
                "wqk": np.ascontiguousarray(wqk).astype(NPBF16),
                "wvT": np.ascontiguousarray(wvT).astype(NPBF16),
                "woT": np.ascontiguousarray(woT).astype(NPBF16),
            }
        )
    return in_maps


def fold_weights(Wq, Wk, Wv, Wo, Aq, Bq, Ak, Bk, Av, Bv, Ao, Bo):
    scale = 1.0 / np.sqrt(np.float32(D))
    Wq_eff = (Wq + Bq @ Aq) * scale
    Wk_eff = Wk + Bk @ Ak
    Wv_eff = Wv + Bv @ Av
    Wo_eff = Wo + Bo @ Ao
    return Wq_eff, Wk_eff, Wv_eff, Wo_eff


_NC_CACHE = {}


def _get_program(S, C):
    key = (S, C)
    if key not in _NC_CACHE:
        _NC_CACHE[key] = build_program(S, C)
    return _NC_CACHE[key]


def kernel(**inputs):
    inputs = {k: np.asarray(v, np.float32) for k, v in inputs.items()}
    x = inputs["x"]
    B, S, C = x.shape
    Wq_eff, Wk_eff, Wv_eff, Wo_eff = fold_weights(
        inputs["Wq"], inputs["Wk"], inputs["Wv"], inputs["Wo"],
        inputs["Aq"], inputs["Bq"], inputs["Ak"], inputs["Bk"],
        inputs["Av"], inputs["Bv"], inputs["Ao"], inputs["Bo"],
    )
    in_maps = make_core_inputs(x, Wq_eff, Wk_eff, Wv_eff, Wo_eff)
    nc = _get_program(S, C)
    res = run_bass_kernel_spmd(nc, in_maps, list(range(N_CORES)))
    parts = [res.results[c]["out_part"].astype(np.float32) for c in range(N_CORES)]
    bo = inputs["bo"]
    out = np.stack(
        [
            parts[0] + parts[1] + parts[2] + parts[3] + bo,
            parts[4] + parts[5] + parts[6] + parts[7] + bo,
        ]
    ).astype(np.float32)
    return out
